# revision 4
# baseline (speedup 1.0000x reference)
"""Batched GAT (dense adjacency) Trainium2 Bass kernel — rank-1 attention.

Key idea: softmax over sources j is invariant to scaling column i of the
transposed score matrix, so with es/ed the per-head source/dest logits:

    exp(prelu(es_j + ed_i)) ~_i  max(exp(es_j), exp(-0.8*ed_i)*exp(0.2*es_j))

This removes every N^2-scale activation: per (head, source-chunk) tile the
scores become ONE DVE tensor_scalar ((R_bc * D_col) max B_col, per-partition
scalars) plus ONE mask multiply (DVE).  exp() runs only on O(N) vectors
(ACT).  The adjacency mask is thresholded and transposed on the host, so the
kernel has ZERO PE transposes.  The aggregation appends a ones-column to h so
the softmax denominators fall out of the same PE matmuls, and arrive
per-node-partition so no transpose is needed for the reciprocal.

DVE is the bottleneck engine (~98% busy), so everything movable is elsewhere:
  - the output bias rides the h matmul as a K=2 ones-row PSUM accumulation
    (out = attn@(h+bias) = attn@h + bias because attn rows sum to 1), which
    also removes the per-chunk fp32 bias adds from DVE;
  - softmax denominators are staged PSUM->SBUF by ACT so one batched DVE
    reciprocal per head replaces 2 strided PSUM reciprocals (120-cycle PSUM
    init each).

Sharding: batch (B=16) across 8 cores, 2 samples/core, weights replicated.
"""

import numpy as np

import concourse.bacc as bacc
import concourse.tile as tile
from concourse import mybir
from concourse.bass_utils import run_bass_kernel_spmd

F32 = mybir.dt.float32
BF16 = mybir.dt.bfloat16
AF = mybir.ActivationFunctionType
ALU = mybir.AluOpType

P = 128          # partitions
N = 1024         # nodes
D = 256          # input feature dim
H = 4            # heads
F = 64           # per-head dim
HF = H * F       # 256
NCH = N // P     # 8 node chunks
NCORES = 8
BPC = 2          # batch samples per core
DW = D + 2 * H   # fused h+e matmul moving width (264)
FP1 = F + 1      # per-head agg output width (features + denominator)


def build_nc(num_devices=NCORES, repeat=1):
    nc = bacc.Bacc("TRN2", target_bir_lowering=False, debug=False,
                   num_devices=num_devices)
    xt_d = nc.dram_tensor("xt", [BPC, D, N], BF16, kind="ExternalInput")
    adjm_d = nc.dram_tensor("adjm", [BPC, N, N], BF16, kind="ExternalInput")
    waug_d = nc.dram_tensor("waug", [D, DW], BF16, kind="ExternalInput")
    wad_d = nc.dram_tensor("wad", [D, H], BF16, kind="ExternalInput")
    biasx_d = nc.dram_tensor("biasx", [2, DW], BF16, kind="ExternalInput")
    ones2_d = nc.dram_tensor("ones2", [2, P], BF16, kind="ExternalInput")
    out_d = nc.dram_tensor("out", [BPC, N, HF], F32, kind="ExternalOutput")

    with tile.TileContext(nc) as tc:
        with (
            tc.tile_pool(name="consts", bufs=1) as consts,
            tc.tile_pool(name="w4", bufs=4) as p_w4,
            tc.tile_pool(name="pm4", bufs=6) as p_pm4,
            tc.tile_pool(name="recip", bufs=6) as p_recip,
            tc.tile_pool(name="ot", bufs=2) as p_ot,
            tc.tile_pool(name="ps", bufs=8, space="PSUM") as p_ps,
            tc.tile_pool(name="dram", bufs=2, space="DRAM") as p_dram,
        ):
            waug_sb = consts.tile([P, 2, DW], BF16)
            for dc in range(2):
                nc.sync.dma_start(waug_sb[:, dc, :], waug_d[dc * P:(dc + 1) * P, :])
            wad_sb = consts.tile([P, 2, H], BF16)
            for dc in range(2):
                nc.sync.dma_start(wad_sb[:, dc, :], wad_d[dc * P:(dc + 1) * P, :])
            # bias folded into the h matmul as a K=2 ones-row contribution
            # (row 1 of both operands is zero padding; K=1 loads fail on HW):
            # out = (sum_j pm (h+bias))/denom = attn@h + bias since attn sums
            # to 1; the e-logit columns get +0 (biasx[:, D:] == 0).
            biasx_sb = consts.tile([2, DW], BF16)
            nc.sync.dma_start(biasx_sb[:], biasx_d[:])
            ones1 = consts.tile([2, P], BF16)
            nc.sync.dma_start(ones1[:], ones2_d[:])
            haug_bufs = [consts.tile([P, NCH, H, FP1], BF16, name=f"haugb{b}")
                         for b in range(2)]
            for hb in haug_bufs:
                nc.vector.memset(hb[:], 1.0)
            xt_bufs = [consts.tile([P, 2, N], BF16, name=f"xtb{b}")
                       for b in range(2)]
            mask_bufs = [consts.tile([P, NCH, N], BF16, name=f"maskb{b}")
                         for b in range(2)]
            rbc_bufs = [consts.tile([P, H, N], BF16, name=f"rbcb{b}")
                        for b in range(2)]
            eb_bufs = [consts.tile([P, NCH, 2 * H], F32, name=f"ebb{b}")
                       for b in range(2)]
            ed_bufs = [consts.tile([P, NCH, 2 * H], F32, name=f"edb{b}")
                       for b in range(2)]
            r4_bufs = [consts.tile([H, N], BF16, name=f"r4b{b}")
                       for b in range(2)]
            scr_bufs = [p_dram.tile([H, N], BF16, tag="scr", name=f"scr{b}")
                        for b in range(2)]

            def phase_pre(b):
                """Generator preamble for sample b; yields between chunks so
                it can interleave with the previous sample's attention."""
                xt_t = xt_bufs[b]
                nc.sync.dma_start(
                    xt_t[:, :, :],
                    xt_d[b].rearrange("(c p) n -> p c n", p=P))
                xtr = xt_t
                yield

                haug_t = haug_bufs[b]
                expB = eb_bufs[b]
                expD = ed_bufs[b]
                yield

                # dest-logit rows -> R = exp(-0.8*ed), broadcast via DRAM.
                # Issued first (needs only xt) so the DRAM roundtrip overlaps
                # the h chunks instead of stalling the next attention phase.
                r4 = r4_bufs[b]
                for nh in range(2):
                    per = p_ps.tile([H, 512], F32, tag="u", name=f"per{b}{nh}")
                    for dc in range(2):
                        nc.tensor.matmul(per[:],
                                         wad_sb[:, dc, :],
                                         xtr[:, dc, nh * 512:(nh + 1) * 512],
                                         start=(dc == 0), stop=(dc == 1))
                    nc.scalar.activation(out=r4[:, nh * 512:(nh + 1) * 512],
                                         in_=per[:], func=AF.Exp, scale=-0.8)
                yield
                scr = scr_bufs[b]
                nc.sync.dma_start(scr[:], r4[:])
                rbc = rbc_bufs[b]
                for h in range(H):
                    nc.sync.dma_start(rbc[:, h, :],
                                      scr[h, :].partition_broadcast(P))
                yield

                # transposed 0/1 edge mask, thresholded on the host
                maskT = mask_bufs[b]
                av = adjm_d[b].rearrange("(c p) i -> p c i", p=P)
                for qt in range(4):
                    nc.sync.dma_start(maskT[:, qt * 2:(qt + 1) * 2, :],
                                      av[:, qt * 2:(qt + 1) * 2, :])
                    yield

                # h (+ fused e-logit columns); col F stays 1.0 (preset once)
                for ic in range(NCH):
                    ph = p_ps.tile([P, 512], F32, tag="u", name=f"ph{b}{ic}")
                    for dc in range(2):
                        nc.tensor.matmul(ph[:, 0:DW],
                                         xtr[:, dc, ic * P:(ic + 1) * P],
                                         waug_sb[:, dc, :],
                                         start=(dc == 0), stop=False)
                    nc.tensor.matmul(ph[:, 0:DW], ones1[:, :], biasx_sb[:, :],
                                     start=False, stop=True)
                    nc.scalar.activation(
                        out=haug_t[:, ic, :, 0:F],
                        in_=ph[:, 0:D].rearrange("p (h f) -> p h f", h=H),
                        func=AF.Copy)
                    nc.scalar.activation(out=expB[:, ic, :], in_=ph[:, D:DW],
                                         func=AF.Exp)
                    nc.scalar.activation(out=expD[:, ic, :], in_=ph[:, D:DW],
                                         func=AF.Exp, scale=0.2)
                    yield

                yield (haug_t, expB, expD, rbc, maskT)

            def run_pre(b):
                st = None
                for st in phase_pre(b):
                    pass
                return st

            def phase_att(b, state, interleave=None):
                haug_t, expB, expD, rbc, maskT = state
                recip_t = p_recip.tile([P, H, NCH], F32, tag="recip",
                                       name=f"rc{b}")
                ot = p_ot.tile([P, NCH, HF], F32, tag="ot", name=f"ot{b}")
                nxt = None

                def make_w4(h, q):
                    w4 = p_w4.tile([P, 4, N], BF16, tag="w4",
                                   name=f"w4{b}_{h}{q}")
                    for k in range(4):
                        jc = q * 4 + k
                        nc.vector.tensor_scalar(
                            out=w4[:, k, :], in0=rbc[:, h, :],
                            scalar1=expD[:, jc, 2 * h:2 * h + 1],
                            scalar2=expB[:, jc, 2 * h:2 * h + 1],
                            op0=ALU.mult, op1=ALU.max)
                    return w4

                def make_pm4(h, q):
                    w4 = make_w4(h, q)
                    pm4 = p_pm4.tile([P, 4, N], BF16, tag="pm4",
                                     name=f"pm4{b}_{h}{q}")
                    nc.vector.tensor_tensor(out=pm4[:], in0=w4[:],
                                            in1=maskT[:, q * 4:(q + 1) * 4, :],
                                            op=ALU.mult)
                    return pm4

                for h in range(H):
                    bks = (p_ps.tile([P, 512], F32, tag="u", name=f"agA{b}{h}"),
                           p_ps.tile([P, 512], F32, tag="u", name=f"agB{b}{h}"))
                    for q in range(2):
                        pm4 = make_pm4(h, q)
                        for k in range(4):
                            jc = q * 4 + k
                            for ic in range(NCH):
                                bk = bks[ic // 4]
                                g = ic % 4
                                nc.tensor.matmul(
                                    bk[:, g * FP1:(g + 1) * FP1],
                                    pm4[:, k, ic * P:(ic + 1) * P],
                                    haug_t[:, jc, h, :],
                                    start=(jc == 0 and g == 0),
                                    stop=(jc == NCH - 1 and g == 3))
                        if interleave is not None:
                            nxt = next(interleave, nxt)

                    # stage denominators PSUM->SBUF on ACT so one batched DVE
                    # reciprocal covers all 8 chunks (PSUM-src recips pay a
                    # 120-cycle init each; the SBUF batch pays it once)
                    den = p_recip.tile([P, NCH], F32, tag="den",
                                       name=f"dn{b}{h}")
                    for s in range(2):
                        bkv = bks[s][:, 0:4 * FP1].rearrange(
                            "p (g c) -> p g c", c=FP1)
                        nc.scalar.activation(out=den[:, s * 4:(s + 1) * 4],
                                             in_=bkv[:, 0:4, F], func=AF.Copy)
                    nc.vector.reciprocal(recip_t[:, h, :], den[:, :])
                    for s in range(2):
                        bkv = bks[s][:, 0:4 * FP1].rearrange(
                            "p (g c) -> p g c", c=FP1)
                        for g in range(4):
                            ic = s * 4 + g
                            nc.scalar.activation(
                                out=ot[:, ic, h * F:(h + 1) * F],
                                in_=bkv[:, g, 0:F],
                                func=AF.Copy,
                                scale=recip_t[:, h, ic:ic + 1])
                        if interleave is not None:
                            nxt = next(interleave, nxt)

                nc.sync.dma_start(
                    out_d[b].rearrange("(c p) f -> p c f", p=P),
                    ot[:, :, :])
                return nxt

            ST = [None, None]

            def piped(b_att, b_pre):
                gen = phase_pre(b_pre)
                phase_att(b_att, ST[b_att], interleave=gen)
                for last in gen:
                    ST[b_pre] = last

            gen0 = phase_pre(0)
            for last in gen0:
                ST[0] = last
            if repeat == 1:
                piped(0, 1)
                phase_att(1, ST[1])
            elif repeat <= 4:
                # fully unrolled, no For_i (also what TimelineSim can model)
                for _ in range(repeat):
                    piped(0, 1)
                    piped(1, 0)
            else:
                # Software pipeline with manual unroll: For_i carries an
                # all-engine barrier per iteration, so amortize it over U
                # bodies; persistent per-sample buffers keep the trace-time
                # handles valid across iterations.
                U = max(u for u in (16, 8, 4, 2, 1) if repeat % u == 0)
                with tc.For_i(0, repeat // U, 1):
                    for _ in range(U):
                        piped(0, 1)
                        piped(1, 0)

    nc.compile()
    return nc


_NC_CACHE = {}


def _get_nc():
    if "nc" not in _NC_CACHE:
        _NC_CACHE["nc"] = build_nc()
    return _NC_CACHE["nc"]


def _prep_weights(W, a_src, a_dst, bias):
    W2 = np.ascontiguousarray(np.asarray(W).reshape(D, HF)).astype(np.float32)
    acat = np.zeros((HF, 2 * H), np.float32)
    a_src = np.asarray(a_src, np.float32)
    a_dst = np.asarray(a_dst, np.float32)
    for h in range(H):
        acat[h * F:(h + 1) * F, 2 * h] = a_src[h]
        acat[h * F:(h + 1) * F, 2 * h + 1] = a_dst[h]
    import ml_dtypes
    wa = W2 @ acat                                           # [D, 2H]
    waug = np.ascontiguousarray(
        np.concatenate([W2, wa], axis=1).astype(ml_dtypes.bfloat16))
    wad = np.ascontiguousarray(wa[:, 1::2].astype(ml_dtypes.bfloat16))
    biasx = np.zeros((2, DW), np.float32)
    biasx[0, :HF] = np.asarray(bias, np.float32)
    biasx = np.ascontiguousarray(biasx.astype(ml_dtypes.bfloat16))
    ones2 = np.zeros((2, P), np.float32)
    ones2[0, :] = 1.0
    ones2 = np.ascontiguousarray(ones2.astype(ml_dtypes.bfloat16))
    return waug, wad, biasx, ones2


def prep_core_inputs(x, adj, W, a_src, a_dst, bias):
    """Host-side layout prep: per-core shards with x and adj pre-transposed."""
    x = np.asarray(x, np.float32)
    adj = np.asarray(adj, np.float32)
    waug, wad, biasx, ones2 = _prep_weights(W, a_src, a_dst, bias)
    in_maps = []
    for c in range(NCORES):
        xs = x[c * BPC:(c + 1) * BPC]
        as_ = adj[c * BPC:(c + 1) * BPC]
        import ml_dtypes
        in_maps.append({
            "xt": np.ascontiguousarray(
                xs.transpose(0, 2, 1).astype(ml_dtypes.bfloat16)),
            "adjm": np.ascontiguousarray(
                (as_.transpose(0, 2, 1) > 0.5).astype(ml_dtypes.bfloat16)),
            "waug": waug, "wad": wad, "biasx": biasx, "ones2": ones2,
        })
    return in_maps


def kernel(x, adj, W, a_src, a_dst, bias):
    in_maps = prep_core_inputs(x, adj, W, a_src, a_dst, bias)
    nc = _get_nc()
    r = run_bass_kernel_spmd(nc, in_maps, core_ids=list(range(NCORES)))
    return np.concatenate([r.results[c]["out"] for c in range(NCORES)], axis=0)
